# revision 15
# baseline (speedup 1.0000x reference)
"""Trainium2 Bass kernel for nn_MultiHeadAttention_75771813036481.

Reference semantics (note the faithful source bug):
    q/k/v proj  -> reshape(B, H, S, DK) WITHOUT head transpose (heads
                   partition the flattened (S*D) buffer)
    score = QK^T/sqrt(DK); attn = softmax(score)  [returned]
    out   = score @ V     (pre-softmax score!)  -> concat -> @ Wc.T + bc

Because `out` uses the *pre-softmax* score, it reassociates:
    out_h = Q_h @ ((K_h^T V_h)/8)   with (K_h^T V_h) a tiny [64,64] matrix.
The 400MB attn tensor is the memory-bound part.

Sharding: 24 (b,h) pairs over 8 cores, 3 heads per core. A triple of
heads spans exactly 512 consecutive sequence rows of the projected
buffer (3*2048 g-rows = 512 s-rows), so core c needs only
q/k/v[b, 512*cg:512*cg+512, :] with b=c//4, cg=c%4.

Layouts on device (g = "group row" index = 12*s' + j, j = feature block):
    Qt  [64, 6144]   Qt[d, g]  = Q_proj[s'(g), 64*j(g)+d]   (3 heads concat)
    KVt [128, 6144]  rows 0:64 = Kt, rows 64:128 = Vt
Projections are computed j-grouped (lhsT = W^T column block) so the
g-interleave is just a stride-12 PSUM->SBUF copy.
"""

import numpy as np

B, S, D, H = 2, 2048, 768, 12
DK = 64
NJ = 12          # feature blocks per row (D // DK)
SLAB = 512       # s-rows per core
GC = 6144        # g-columns per core (3 heads * 2048)
N_CORES = 8

_BASS_CACHE = {}
_RUN_KWARGS = {}  # test harness may set {"trace": True} etc.


def _build_bass(phases=4):
    import concourse.mybir as mybir
    import concourse.tile as tile
    from concourse import bacc

    f32 = mybir.dt.float32
    f32r = mybir.dt.float32r
    AF = mybir.ActivationFunctionType

    nc = bacc.Bacc(name="mha_bug_attn")

    # ---- I/O ----
    qT = nc.dram_tensor("qT", [D, SLAB], f32r, kind="ExternalInput")
    kT = nc.dram_tensor("kT", [D, SLAB], f32r, kind="ExternalInput")
    vT = nc.dram_tensor("vT", [D, SLAB], f32r, kind="ExternalInput")
    wqT = nc.dram_tensor("wqT", [D, D], f32r, kind="ExternalInput")
    wkT = nc.dram_tensor("wkT", [D, D], f32r, kind="ExternalInput")
    wvT = nc.dram_tensor("wvT", [D, D], f32r, kind="ExternalInput")
    wcTh = nc.dram_tensor("wcTh", [3 * DK, D], f32r, kind="ExternalInput")
    bq_t = nc.dram_tensor("bq_t", [DK, NJ], f32, kind="ExternalInput")
    bkv_t = nc.dram_tensor("bkv_t", [2 * DK, NJ], f32, kind="ExternalInput")
    bc_b = nc.dram_tensor("bc_b", [128, D], f32, kind="ExternalInput")
    identD = nc.dram_tensor("identD", [128, 128], f32r, kind="ExternalInput")

    attn_o = nc.dram_tensor("attn_o", [3, S, S], f32, kind="ExternalOutput")
    outp_o = nc.dram_tensor("outp_o", [S, D], f32, kind="ExternalOutput")

    with tile.TileContext(nc) as tc:
        with tc.tile_pool(name="singles", bufs=1) as singles:
            ident = singles.tile([128, 128], f32r)
            nc.sync.dma_start(ident[:], identD[:])
            bq_sb = singles.tile([DK, NJ], f32)
            nc.sync.dma_start(bq_sb[:], bq_t[:])
            bkv_sb = singles.tile([2 * DK, NJ], f32)
            nc.sync.dma_start(bkv_sb[:], bkv_t[:])
            bc_sb = singles.tile([128, D], f32)
            nc.sync.dma_start(bc_sb[:], bc_b[:])
            wcTh_sb = singles.tile([DK, 3, D], f32r)
            nc.sync.dma_start(wcTh_sb[:], wcTh[:].rearrange("(h d) o -> d h o", d=DK))

            Qt = singles.tile([DK, GC], f32r)
            Kt = singles.tile([DK, GC], f32r)
            Vt = singles.tile([DK, GC], f32r)

            # f32r matmuls do not trip the PE HAM activity counter, so the
            # clock gate stays at 1.2GHz unless bf16 work warms it. Dummy
            # bf16 matmul bursts keep the PE at 2.4GHz.
            bf16 = mybir.dt.bfloat16
            warm_w = singles.tile([128, 128], bf16)
            warm_x = singles.tile([128, 512], bf16)
            nc.vector.memset(warm_w[:], 0.0)
            nc.vector.memset(warm_x[:], 0.0)

            def warm_burst(pool, n, tag="warm_ps", cols=512):
                wp = pool.tile([128, 512], f32, tag=tag)
                for _ in range(n):
                    nc.tensor.matmul(wp[:, :cols], warm_w[:], warm_x[:, :cols],
                                     start=True, stop=True)

            # ---------------- phase 1: projections ----------------
            with (
                tc.tile_pool(name="ph1", bufs=1) as ph1,
                tc.tile_pool(name="ph1ps", bufs=4, space="PSUM") as ph1ps,
            ):
                wq_sb = ph1.tile([128, 6, D], f32r)
                nc.sync.dma_start(wq_sb[:], wqT[:].rearrange("(co p) o -> p co o", p=128))
                wk_sb = ph1.tile([128, 6, D], f32r)
                nc.sync.dma_start(wk_sb[:], wkT[:].rearrange("(co p) o -> p co o", p=128))
                wv_sb = ph1.tile([128, 6, D], f32r)
                nc.sync.dma_start(wv_sb[:], wvT[:].rearrange("(co p) o -> p co o", p=128))
                qT_sb = ph1.tile([128, 6, SLAB], f32r)
                nc.sync.dma_start(qT_sb[:], qT[:].rearrange("(co p) s -> p co s", p=128))
                kT_sb = ph1.tile([128, 6, SLAB], f32r)
                nc.sync.dma_start(kT_sb[:], kT[:].rearrange("(co p) s -> p co s", p=128))
                vT_sb = ph1.tile([128, 6, SLAB], f32r)
                nc.sync.dma_start(vT_sb[:], vT[:].rearrange("(co p) s -> p co s", p=128))

                warm_burst(ph1ps, 48)

                # paired j: lhsT takes two adjacent 64-col weight blocks
                # (M=128); the odd j's rows land on partitions 64:128 and the
                # strided copy shifts them down (DVE handles partition-base
                # mismatch).
                for jp in range(6 if phases >= 1 else 0):
                    warm_burst(ph1ps, 2, cols=256)
                    j0, j1 = 2 * jp, 2 * jp + 1
                    js = slice(128 * jp, 128 * jp + 128)
                    for name, w_sb, x_sb, dst, bias in (
                        ("q", wq_sb, qT_sb, Qt, bq_sb),
                        ("k", wk_sb, kT_sb, Kt, bkv_sb[0:DK]),
                        ("v", wv_sb, vT_sb, Vt, bkv_sb[DK:128]),
                    ):
                        ps_p = ph1ps.tile([128, SLAB], f32, tag="ps1",
                                          name=f"ps_{name}{jp}")
                        for co in range(6):
                            nc.tensor.matmul(
                                ps_p[:], w_sb[:, co, js], x_sb[:, co, :],
                                start=(co == 0), stop=(co == 5),
                            )
                        nc.vector.tensor_scalar_add(
                            dst[:, j0:GC:NJ], ps_p[0:DK, :], bias[:, j0:j0 + 1]
                        )
                        nc.vector.tensor_scalar_add(
                            dst[:, j1:GC:NJ], ps_p[DK:128, :], bias[:, j1:j1 + 1]
                        )

            # ---------------- phase 2: per-head attention ----------------
            with (
                tc.tile_pool(name="persist2", bufs=1) as persist2,
                tc.tile_pool(name="work", bufs=2) as work,
                tc.tile_pool(name="attn_pool", bufs=3) as attn_pool,
                tc.tile_pool(name="small", bufs=4) as small,
                tc.tile_pool(name="ps_misc", bufs=2, space="PSUM") as ps_misc,
                tc.tile_pool(name="score_ps", bufs=3, space="PSUM") as score_ps,
            ):
                outhT = persist2.tile([DK, 3, S], f32r)
                for hh in range(3 if phases >= 2 else 0):
                    C = S * hh
                    warm_burst(ps_misc, 8, tag="ps2")
                    # KV_nat[g, d|e] via PE transpose of KVt column chunks
                    KVnat = work.tile([128, S], f32r, tag="kvnat")
                    for tq in range(4):
                        ps_tr = ps_misc.tile([128, 512], f32r, tag="ps2")
                        for u in range(4):
                            t = 4 * tq + u
                            nc.tensor.transpose(
                                ps_tr[:, 128 * u:128 * u + 64],
                                Kt[0:DK, C + 128 * t:C + 128 * t + 128],
                                ident[0:DK, 0:DK],
                            )
                            nc.tensor.transpose(
                                ps_tr[:, 128 * u + 64:128 * u + 128],
                                Vt[0:DK, C + 128 * t:C + 128 * t + 128],
                                ident[0:DK, 0:DK],
                            )
                        nc.vector.tensor_copy(
                            KVnat[:, 512 * tq:512 * tq + 512], ps_tr[:]
                        )
                    warm_burst(ps_misc, 2, tag="ps2", cols=256)
                    # M_h = (1/8) K_h^T V_h  [64, 64]
                    ps_m = ps_misc.tile([128, 512], f32, tag="ps2")
                    for t in range(16):
                        nc.tensor.matmul(
                            ps_m[0:DK, 0:DK],
                            KVnat[:, 128 * t:128 * t + DK],
                            KVnat[:, 128 * t + DK:128 * t + 128],
                            start=(t == 0), stop=(t == 15),
                        )
                    M_sb = small.tile([DK, DK], f32r, tag="msb")
                    nc.scalar.mul(M_sb[:], ps_m[0:DK, 0:DK], 0.125)
                    # out_hT = M_h^T @ Qt_head  [64, 2048]
                    for nn in range(4):
                        ps_o = ps_misc.tile([128, 512], f32, tag="ps2")
                        nc.tensor.matmul(
                            ps_o[0:DK, :], M_sb[:],
                            Qt[:, C + 512 * nn:C + 512 * nn + 512],
                            start=True, stop=True,
                        )
                        nc.vector.tensor_copy(
                            outhT[:, hh, 512 * nn:512 * nn + 512], ps_o[0:DK, :]
                        )
                    # score rows + softmax; two m-chunks share one SBUF
                    # tile so each attn DMA writes 2MB
                    for mp in range(8 if phases >= 3 else 0):
                        warm_burst(ps_misc, 2, tag="ps2", cols=256)
                        att = attn_pool.tile([128, 2 * S], f32, tag="att")
                        for sub in range(2):
                            mi = 2 * mp + sub
                            sums = small.tile([128, 2], f32, tag="sums")
                            for half in range(2):
                                ps_s = score_ps.tile([128, 1024], f32, tag="sc")
                                for nn in range(2):
                                    nc.tensor.matmul(
                                        ps_s[:, 512 * nn:512 * nn + 512],
                                        Qt[:, C + 128 * mi:C + 128 * mi + 128],
                                        Kt[0:DK,
                                           C + 1024 * half + 512 * nn:
                                           C + 1024 * half + 512 * nn + 512],
                                        start=True, stop=True,
                                    )
                                nc.scalar.activation(
                                    att[:, S * sub + 1024 * half:
                                        S * sub + 1024 * half + 1024],
                                    ps_s[:],
                                    AF.Exp,
                                    scale=0.125,
                                    accum_out=sums[:, half:half + 1],
                                )
                            rsum = small.tile([128, 1], f32, tag="rsum")
                            nc.vector.tensor_add(rsum[:], sums[:, 0:1],
                                                 sums[:, 1:2])
                            nc.vector.reciprocal(rsum[:], rsum[:])
                            nc.vector.tensor_scalar_mul(
                                att[:, S * sub:S * sub + S],
                                att[:, S * sub:S * sub + S], rsum[:]
                            )
                        nc.sync.dma_start(
                            attn_o[hh, 256 * mp:256 * mp + 256, :]
                            .rearrange("(m p) n -> p m n", p=128),
                            att[:].rearrange("p (m n) -> p m n", m=2),
                        )

                # ---------------- phase 3: output projection (partial) ----------------
                if phases >= 4:
                    warm_burst(ps_misc, 8, tag="ps2")
                for mi in range(16 if phases >= 4 else 0):
                    if mi % 4 == 0:
                        warm_burst(ps_misc, 2, tag="ps2", cols=256)
                    op_t = work.tile([128, D], f32, tag="outp")
                    for half in range(2):
                        ps_f = ps_misc.tile([128, 512], f32, tag="ps2")
                        for hh2 in range(3):
                            nc.tensor.matmul(
                                ps_f[:, 0:384],
                                outhT[:, hh2, 128 * mi:128 * mi + 128],
                                wcTh_sb[:, hh2, 384 * half:384 * half + 384],
                                start=(hh2 == 0), stop=(hh2 == 2),
                            )
                        nc.vector.tensor_add(
                            op_t[:, 384 * half:384 * half + 384],
                            ps_f[:, 0:384],
                            bc_sb[:, 384 * half:384 * half + 384],
                        )
                    nc.sync.dma_start(outp_o[128 * mi:128 * mi + 128, :], op_t[:])

    nc.compile()
    return nc


def _get_bass():
    if "nc" not in _BASS_CACHE:
        _BASS_CACHE["nc"] = _build_bass()
    return _BASS_CACHE["nc"]


def kernel(q, k, v, Wq, bq, Wk, bk, Wv, bv, Wc, bc, **_unused):
    from concourse.bass_utils import run_bass_kernel_spmd

    q = np.asarray(q, np.float32)
    k = np.asarray(k, np.float32)
    v = np.asarray(v, np.float32)
    Wq = np.asarray(Wq, np.float32)
    Wk = np.asarray(Wk, np.float32)
    Wv = np.asarray(Wv, np.float32)
    Wc = np.asarray(Wc, np.float32)
    bq = np.asarray(bq, np.float32)
    bk = np.asarray(bk, np.float32)
    bv = np.asarray(bv, np.float32)
    bc = np.asarray(bc, np.float32)

    WqT = np.ascontiguousarray(Wq.T)
    WkT = np.ascontiguousarray(Wk.T)
    WvT = np.ascontiguousarray(Wv.T)
    WcT = np.ascontiguousarray(Wc.T)
    bq_t = np.ascontiguousarray(bq.reshape(NJ, DK).T)
    bkv_t = np.ascontiguousarray(
        np.concatenate([bk.reshape(NJ, DK).T, bv.reshape(NJ, DK).T], axis=0)
    )
    bc_tile = np.ascontiguousarray(np.tile(bc[None, :], (128, 1)))
    bc_zero = np.zeros_like(bc_tile)
    ident = np.eye(128, dtype=np.float32)

    in_maps = []
    for c in range(N_CORES):
        b, cg = divmod(c, 4)
        s0 = SLAB * cg
        in_maps.append({
            "qT": np.ascontiguousarray(q[b, s0:s0 + SLAB, :].T),
            "kT": np.ascontiguousarray(k[b, s0:s0 + SLAB, :].T),
            "vT": np.ascontiguousarray(v[b, s0:s0 + SLAB, :].T),
            "wqT": WqT,
            "wkT": WkT,
            "wvT": WvT,
            "wcTh": np.ascontiguousarray(WcT[192 * cg:192 * cg + 192, :]),
            "bq_t": bq_t,
            "bkv_t": bkv_t,
            # bc must be added exactly once per batch: only cg==0 cores add it
            "bc_b": bc_tile if cg == 0 else bc_zero,
            "identD": ident,
        })

    nc = _get_bass()
    res = run_bass_kernel_spmd(
        nc, in_maps, core_ids=list(range(N_CORES)), **_RUN_KWARGS
    )
    _BASS_CACHE["last_result"] = res

    attn = np.empty((B, H, S, S), np.float32)
    out = np.zeros((B, S, D), np.float32)
    for c in range(N_CORES):
        b, cg = divmod(c, 4)
        attn[b, 3 * cg:3 * cg + 3] = res.results[c]["attn_o"]
        out[b] += res.results[c]["outp_o"]
    return out, attn


# revision 19
# speedup vs baseline: 1.0366x; 1.0366x over previous
"""Trainium2 Bass kernel for nn_MultiHeadAttention_75771813036481.

Reference semantics (note the faithful source bug):
    q/k/v proj  -> reshape(B, H, S, DK) WITHOUT head transpose (heads
                   partition the flattened (S*D) buffer)
    score = QK^T/sqrt(DK); attn = softmax(score)  [returned]
    out   = score @ V     (pre-softmax score!)  -> concat -> @ Wc.T + bc

Because `out` uses the *pre-softmax* score, it reassociates:
    out_h = Q_h @ ((K_h^T V_h)/8)   with (K_h^T V_h) a tiny [64,64] matrix.
The 400MB attn tensor is the memory-bound part.

Sharding: 24 (b,h) pairs over 8 cores, 3 heads per core. A triple of
heads spans exactly 512 consecutive sequence rows of the projected
buffer (3*2048 g-rows = 512 s-rows), so core c needs only
q/k/v[b, 512*cg:512*cg+512, :] with b=c//4, cg=c%4.

Layouts on device (g = "group row" index = 12*s' + j, j = feature block):
    Qt  [64, 6144]   Qt[d, g]  = Q_proj[s'(g), 64*j(g)+d]   (3 heads concat)
    KVt [128, 6144]  rows 0:64 = Kt, rows 64:128 = Vt
Projections are computed j-grouped (lhsT = W^T column block) so the
g-interleave is just a stride-12 PSUM->SBUF copy.
"""

import numpy as np

B, S, D, H = 2, 2048, 768, 12
DK = 64
NJ = 12          # feature blocks per row (D // DK)
SLAB = 512       # s-rows per core
GC = 6144        # g-columns per core (3 heads * 2048)
N_CORES = 8

_BASS_CACHE = {}
_RUN_KWARGS = {}  # test harness may set {"trace": True} etc.


def _build_bass(phases=4):
    import concourse.mybir as mybir
    import concourse.tile as tile
    from concourse import bacc

    f32 = mybir.dt.float32
    f32r = mybir.dt.float32r
    AF = mybir.ActivationFunctionType

    nc = bacc.Bacc(name="mha_bug_attn")

    # ---- I/O ----
    qT = nc.dram_tensor("qT", [D, SLAB], f32r, kind="ExternalInput")
    kT = nc.dram_tensor("kT", [D, SLAB], f32r, kind="ExternalInput")
    vT = nc.dram_tensor("vT", [D, SLAB], f32r, kind="ExternalInput")
    wqT = nc.dram_tensor("wqT", [D, D], f32r, kind="ExternalInput")
    wkT = nc.dram_tensor("wkT", [D, D], f32r, kind="ExternalInput")
    wvT = nc.dram_tensor("wvT", [D, D], f32r, kind="ExternalInput")
    wcTh = nc.dram_tensor("wcTh", [3 * DK, D], f32r, kind="ExternalInput")
    bq_t = nc.dram_tensor("bq_t", [DK, NJ], f32, kind="ExternalInput")
    bkv_t = nc.dram_tensor("bkv_t", [2 * DK, NJ], f32, kind="ExternalInput")
    bc_b = nc.dram_tensor("bc_b", [128, D], f32, kind="ExternalInput")
    identD = nc.dram_tensor("identD", [128, 128], f32r, kind="ExternalInput")

    attn_o = nc.dram_tensor("attn_o", [3, S, S], f32, kind="ExternalOutput")
    outp_o = nc.dram_tensor("outp_o", [S, D], f32, kind="ExternalOutput")

    with tile.TileContext(nc) as tc:
        with tc.tile_pool(name="singles", bufs=1) as singles:
            ident = singles.tile([128, 128], f32r)
            nc.sync.dma_start(ident[:], identD[:])
            bq_sb = singles.tile([DK, NJ], f32)
            nc.sync.dma_start(bq_sb[:], bq_t[:])
            bkv_sb = singles.tile([2 * DK, NJ], f32)
            nc.sync.dma_start(bkv_sb[:], bkv_t[:])
            bc_sb = singles.tile([128, D], f32)
            nc.sync.dma_start(bc_sb[:], bc_b[:])
            wcTh_sb = singles.tile([DK, 3, D], f32r)
            nc.sync.dma_start(wcTh_sb[:], wcTh[:].rearrange("(h d) o -> d h o", d=DK))

            Qt = singles.tile([DK, GC], f32r)
            Kt = singles.tile([DK, GC], f32r)
            Vt = singles.tile([DK, GC], f32r)

            # f32r matmuls do not trip the PE HAM activity counter, so the
            # clock gate stays at 1.2GHz unless bf16 work warms it. Dummy
            # bf16 matmul bursts keep the PE at 2.4GHz.
            bf16 = mybir.dt.bfloat16
            warm_w = singles.tile([128, 128], bf16)
            warm_x = singles.tile([128, 512], bf16)
            nc.vector.memset(warm_w[:], 0.0)
            nc.vector.memset(warm_x[:], 0.0)

            def warm_burst(pool, n, tag="warm_ps", cols=512):
                wp = pool.tile([128, 512], f32, tag=tag)
                for _ in range(n):
                    nc.tensor.matmul(wp[:, :cols], warm_w[:], warm_x[:, :cols],
                                     start=True, stop=True)

            # ---------------- phase 1: projections ----------------
            with (
                tc.tile_pool(name="ph1", bufs=1) as ph1,
                tc.tile_pool(name="ph1ps", bufs=4, space="PSUM") as ph1ps,
            ):
                wq_sb = ph1.tile([128, 6, D], f32r)
                nc.sync.dma_start(wq_sb[:], wqT[:].rearrange("(co p) o -> p co o", p=128))
                wk_sb = ph1.tile([128, 6, D], f32r)
                nc.sync.dma_start(wk_sb[:], wkT[:].rearrange("(co p) o -> p co o", p=128))
                wv_sb = ph1.tile([128, 6, D], f32r)
                nc.sync.dma_start(wv_sb[:], wvT[:].rearrange("(co p) o -> p co o", p=128))
                qT_sb = ph1.tile([128, 6, SLAB], f32r)
                nc.sync.dma_start(qT_sb[:], qT[:].rearrange("(co p) s -> p co s", p=128))
                kT_sb = ph1.tile([128, 6, SLAB], f32r)
                nc.sync.dma_start(kT_sb[:], kT[:].rearrange("(co p) s -> p co s", p=128))
                vT_sb = ph1.tile([128, 6, SLAB], f32r)
                nc.sync.dma_start(vT_sb[:], vT[:].rearrange("(co p) s -> p co s", p=128))

                warm_burst(ph1ps, 48)

                # paired j: lhsT takes two adjacent 64-col weight blocks
                # (M=128); the odd j's rows land on partitions 64:128 and the
                # strided copy shifts them down (DVE handles partition-base
                # mismatch).
                for jp in range(6 if phases >= 1 else 0):
                    j0, j1 = 2 * jp, 2 * jp + 1
                    js = slice(128 * jp, 128 * jp + 128)
                    for name, w_sb, x_sb, dst, bias in (
                        ("q", wq_sb, qT_sb, Qt, bq_sb),
                        ("k", wk_sb, kT_sb, Kt, bkv_sb[0:DK]),
                        ("v", wv_sb, vT_sb, Vt, bkv_sb[DK:128]),
                    ):
                        ps_p = ph1ps.tile([128, SLAB], f32, tag="ps1",
                                          name=f"ps_{name}{jp}")
                        for co in range(6):
                            nc.tensor.matmul(
                                ps_p[:], w_sb[:, co, js], x_sb[:, co, :],
                                start=(co == 0), stop=(co == 5),
                            )
                        nc.vector.tensor_scalar_add(
                            dst[:, j0:GC:NJ], ps_p[0:DK, :], bias[:, j0:j0 + 1]
                        )
                        nc.vector.tensor_scalar_add(
                            dst[:, j1:GC:NJ], ps_p[DK:128, :], bias[:, j1:j1 + 1]
                        )

            # ---------------- phase 2: per-head attention ----------------
            with (
                tc.tile_pool(name="persist2", bufs=1) as persist2,
                tc.tile_pool(name="work", bufs=2) as work,
                tc.tile_pool(name="attn_pool", bufs=3) as attn_pool,
                tc.tile_pool(name="small", bufs=4) as small,
                tc.tile_pool(name="ps_misc", bufs=2, space="PSUM") as ps_misc,
                tc.tile_pool(name="score_ps", bufs=3, space="PSUM") as score_ps,
            ):
                outhT = persist2.tile([DK, 3, S], f32r)
                # -- pre-work for all heads: KVnat, M_h, out_hT --
                for hh in range(3 if phases >= 2 else 0):
                    C = S * hh
                    # KV_nat[g, d|e] via PE transpose of KVt column chunks
                    KVnat = work.tile([128, S], f32r, tag="kvnat")
                    for tq in range(4):
                        ps_tr = ps_misc.tile([128, 512], f32r, tag="ps2")
                        for u in range(4):
                            t = 4 * tq + u
                            nc.tensor.transpose(
                                ps_tr[:, 128 * u:128 * u + 64],
                                Kt[0:DK, C + 128 * t:C + 128 * t + 128],
                                ident[0:DK, 0:DK],
                            )
                            nc.tensor.transpose(
                                ps_tr[:, 128 * u + 64:128 * u + 128],
                                Vt[0:DK, C + 128 * t:C + 128 * t + 128],
                                ident[0:DK, 0:DK],
                            )
                        nc.vector.tensor_copy(
                            KVnat[:, 512 * tq:512 * tq + 512], ps_tr[:]
                        )
                    # M_h = (1/8) K_h^T V_h  [64, 64]
                    ps_m = ps_misc.tile([128, 512], f32, tag="ps2")
                    for t in range(16):
                        nc.tensor.matmul(
                            ps_m[0:DK, 0:DK],
                            KVnat[:, 128 * t:128 * t + DK],
                            KVnat[:, 128 * t + DK:128 * t + 128],
                            start=(t == 0), stop=(t == 15),
                        )
                    M_sb = small.tile([DK, DK], f32r, tag="msb")
                    nc.scalar.mul(M_sb[:], ps_m[0:DK, 0:DK], 0.125)
                    # out_hT = M_h^T @ Qt_head  [64, 2048]
                    for nn in range(4):
                        ps_o = ps_misc.tile([128, 512], f32, tag="ps2")
                        nc.tensor.matmul(
                            ps_o[0:DK, :], M_sb[:],
                            Qt[:, C + 512 * nn:C + 512 * nn + 512],
                            start=True, stop=True,
                        )
                        nc.vector.tensor_copy(
                            outhT[:, hh, 512 * nn:512 * nn + 512], ps_o[0:DK, :]
                        )

                # -- final output projection: overlaps the score stream --
                for mi in range(16 if phases >= 4 else 0):
                    op_t = work.tile([128, D], f32, tag="outp")
                    for half in range(2):
                        ps_f = ps_misc.tile([128, 512], f32, tag="ps2")
                        for hh2 in range(3):
                            nc.tensor.matmul(
                                ps_f[:, 0:384],
                                outhT[:, hh2, 128 * mi:128 * mi + 128],
                                wcTh_sb[:, hh2, 384 * half:384 * half + 384],
                                start=(hh2 == 0), stop=(hh2 == 2),
                            )
                        nc.vector.tensor_add(
                            op_t[:, 384 * half:384 * half + 384],
                            ps_f[:, 0:384],
                            bc_sb[:, 384 * half:384 * half + 384],
                        )
                    nc.sync.dma_start(outp_o[128 * mi:128 * mi + 128, :], op_t[:])

                # -- score rows + softmax; two m-chunks share one SBUF
                #    tile so each attn DMA writes 2MB --
                for hh in range(3 if phases >= 2 else 0):
                    C = S * hh
                    for mp in range(8 if phases >= 3 else 0):
                        att = attn_pool.tile([128, 2 * S], f32, tag="att")
                        for sub in range(2):
                            mi = 2 * mp + sub
                            sums = small.tile([128, 2], f32, tag="sums")
                            for half in range(2):
                                ps_s = score_ps.tile([128, 1024], f32, tag="sc")
                                for nn in range(2):
                                    nc.tensor.matmul(
                                        ps_s[:, 512 * nn:512 * nn + 512],
                                        Qt[:, C + 128 * mi:C + 128 * mi + 128],
                                        Kt[0:DK,
                                           C + 1024 * half + 512 * nn:
                                           C + 1024 * half + 512 * nn + 512],
                                        start=True, stop=True,
                                    )
                                nc.scalar.activation(
                                    att[:, S * sub + 1024 * half:
                                        S * sub + 1024 * half + 1024],
                                    ps_s[:],
                                    AF.Exp,
                                    scale=0.125,
                                    accum_out=sums[:, half:half + 1],
                                )
                            rsum = small.tile([128, 1], f32, tag="rsum")
                            nc.vector.tensor_add(rsum[:], sums[:, 0:1],
                                                 sums[:, 1:2])
                            nc.vector.reciprocal(rsum[:], rsum[:])
                            nc.vector.tensor_scalar_mul(
                                att[:, S * sub:S * sub + S],
                                att[:, S * sub:S * sub + S], rsum[:]
                            )
                        nc.sync.dma_start(
                            attn_o[hh, 256 * mp:256 * mp + 256, :]
                            .rearrange("(m p) n -> p m n", p=128),
                            att[:].rearrange("p (m n) -> p m n", m=2),
                        )

    nc.compile()
    return nc


def _get_bass():
    if "nc" not in _BASS_CACHE:
        _BASS_CACHE["nc"] = _build_bass()
    return _BASS_CACHE["nc"]


def kernel(q, k, v, Wq, bq, Wk, bk, Wv, bv, Wc, bc, **_unused):
    from concourse.bass_utils import run_bass_kernel_spmd

    q = np.asarray(q, np.float32)
    k = np.asarray(k, np.float32)
    v = np.asarray(v, np.float32)
    Wq = np.asarray(Wq, np.float32)
    Wk = np.asarray(Wk, np.float32)
    Wv = np.asarray(Wv, np.float32)
    Wc = np.asarray(Wc, np.float32)
    bq = np.asarray(bq, np.float32)
    bk = np.asarray(bk, np.float32)
    bv = np.asarray(bv, np.float32)
    bc = np.asarray(bc, np.float32)

    WqT = np.ascontiguousarray(Wq.T)
    WkT = np.ascontiguousarray(Wk.T)
    WvT = np.ascontiguousarray(Wv.T)
    WcT = np.ascontiguousarray(Wc.T)
    bq_t = np.ascontiguousarray(bq.reshape(NJ, DK).T)
    bkv_t = np.ascontiguousarray(
        np.concatenate([bk.reshape(NJ, DK).T, bv.reshape(NJ, DK).T], axis=0)
    )
    bc_tile = np.ascontiguousarray(np.tile(bc[None, :], (128, 1)))
    bc_zero = np.zeros_like(bc_tile)
    ident = np.eye(128, dtype=np.float32)

    in_maps = []
    for c in range(N_CORES):
        b, cg = divmod(c, 4)
        s0 = SLAB * cg
        in_maps.append({
            "qT": np.ascontiguousarray(q[b, s0:s0 + SLAB, :].T),
            "kT": np.ascontiguousarray(k[b, s0:s0 + SLAB, :].T),
            "vT": np.ascontiguousarray(v[b, s0:s0 + SLAB, :].T),
            "wqT": WqT,
            "wkT": WkT,
            "wvT": WvT,
            "wcTh": np.ascontiguousarray(WcT[192 * cg:192 * cg + 192, :]),
            "bq_t": bq_t,
            "bkv_t": bkv_t,
            # bc must be added exactly once per batch: only cg==0 cores add it
            "bc_b": bc_tile if cg == 0 else bc_zero,
            "identD": ident,
        })

    nc = _get_bass()
    res = run_bass_kernel_spmd(
        nc, in_maps, core_ids=list(range(N_CORES)), **_RUN_KWARGS
    )
    _BASS_CACHE["last_result"] = res

    attn = np.empty((B, H, S, S), np.float32)
    out = np.zeros((B, S, D), np.float32)
    for c in range(N_CORES):
        b, cg = divmod(c, 4)
        attn[b, 3 * cg:3 * cg + 3] = res.results[c]["attn_o"]
        out[b] += res.results[c]["outp_o"]
    return out, attn
